# revision 1
# baseline (speedup 1.0000x reference)
"""MeshGNN Trainium2 kernel.

Mathematical reduction: the reference broadcasts the text projection to all 12
mesh vertices, and the row-normalized kNN adjacency has identical row sums
(every vertex has exactly K_NN=6 neighbors), so node features remain identical
across vertices through every GNN layer.  The whole network therefore
collapses to a per-row MLP:

    h   = relu(x @ W0c + b0c)          W0c = W_text @ (s*W_gnn[0])  (384,256)
    h   = relu(h @ (s*W_gnn[l]) + b_gnn[l])   l = 1..3
    o36 = h @ W4c + b4c                W4c = tile(W_out, 12) (256,36)
    out = o36.reshape(B, 12, 3)        b4c = tile(b_out,12) + template.flat

where s = 6/(6+1e-6) is the common adjacency row sum.

Device strategy (8 cores, pure data parallel over the batch):
  - host pre-transposes each core's x shard to (384, 4096) so features sit on
    SBUF partitions; all matmuls then run in feature-on-partition layout with
    weights as the stationary operand and activations as the moving operand.
  - float32r (default) or bf16 matmuls, 1 PE cycle/row at N=512.
  - relu+bias fused into one op per layer (both m-tiles at once), alternating
    ScalarE / VectorE between layers to balance the two engines.
  - output computed as (36, 4096) on device; host transposes back.
"""

import os

import numpy as np

# ---------------------------------------------------------------- constants
B = 32768
CORES = 8
ROWS = B // CORES            # 4096 rows per core
TD = 384                     # text dim
H = 256                      # hidden
OUT = 36                     # 12 verts * 3 coords
NBLK = 8                     # row blocks per core
N = ROWS // NBLK             # 512 rows per block
KT0 = TD // 128              # 3 k-tiles for layer 0
KTH = H // 128               # 2 k-tiles for hidden layers
MT = H // 128                # 2 m-tiles for hidden outputs
GRP = 4                      # blocks interleaved per scheduling group

MM_DTYPE = os.environ.get("MESHGNN_DTYPE", "f32r")   # "f32r" | "bf16" | "fp8"

_BUILT = {}                  # cache: compiled Bass modules keyed by config


def _np_mm_dtype():
    if MM_DTYPE == "bf16":
        import ml_dtypes
        return ml_dtypes.bfloat16
    if MM_DTYPE == "fp8":
        import concourse.mybir as mybir
        return mybir.dt.np(mybir.dt.float8e4)
    return np.float32


def _build_bass(repeat=1, fake_relu=False, loop_repeat=0, zero_bias=None):
    """Build + compile the per-core Bass program (same NEFF on all cores).

    repeat > 1 re-runs the whole pipeline that many times inside one NEFF
    (identical outputs each pass) -- used for dispatch-free HW timing.
    loop_repeat > 0 wraps the pipeline in a device-side For_i loop executed
    that many times (identical outputs; ~2us barrier per back-edge) -- used
    for timing with enough device work to swamp dispatch noise entirely.
    fake_relu=True makes relu read a constant SBUF tile instead of PSUM
    (wrong results; PE never waits on ACT/DVE) -- PE-floor timing only.
    """
    import concourse.mybir as mybir
    import concourse.tile as tile
    from concourse import bacc

    if zero_bias is None:
        zero_bias = _BUILT.get("zero_bias", False)

    f32 = mybir.dt.float32
    is_fp8 = MM_DTYPE == "fp8"
    if is_fp8:
        mmdt = mybir.dt.float8e4
    elif MM_DTYPE == "bf16":
        mmdt = mybir.dt.bfloat16
    else:
        mmdt = mybir.dt.float32r
    grp = GRP if MM_DTYPE == "fp8" else 2
    DR = mybir.MatmulPerfMode.DoubleRow
    RELU = mybir.ActivationFunctionType.Relu
    IDENT = mybir.ActivationFunctionType.Identity
    ADD = mybir.AluOpType.add
    MAX = mybir.AluOpType.max

    nc = bacc.Bacc(
        "TRN2",
        target_bir_lowering=False,
        debug=False,
        enable_asserts=False,
        num_devices=CORES,
    )

    xt_d = nc.dram_tensor("xt", (TD, ROWS), mmdt, kind="ExternalInput")
    w0_d = nc.dram_tensor("w0", (TD, H), mmdt, kind="ExternalInput")
    wl_d = [
        nc.dram_tensor(f"w{l}", (H, H), mmdt, kind="ExternalInput")
        for l in (1, 2, 3)
    ]
    w4_d = nc.dram_tensor("w4", (H, OUT), mmdt, kind="ExternalInput")
    bl_d = None if zero_bias else [
        nc.dram_tensor(f"b{l}", (128, MT), f32, kind="ExternalInput")
        for l in (0, 1, 2, 3)
    ]
    b4_d = nc.dram_tensor("b4", (OUT, 1), f32, kind="ExternalInput")
    out_d = nc.dram_tensor("out", (OUT, ROWS), f32, kind="ExternalOutput")

    # x viewed as (partition, ktile, row): row-major (TD, ROWS) split over 128
    xt_v = xt_d.ap().rearrange("(k p) n -> p k n", p=128)

    with tile.TileContext(nc) as tc:
        with (
            tc.tile_pool(name="wp", bufs=1) as wp,
            tc.tile_pool(name="xp", bufs=3) as xp,
            tc.tile_pool(name="hp", bufs=3) as hp,
            tc.tile_pool(name="op", bufs=3) as op,
            tc.tile_pool(name="pp", bufs=6, space="PSUM") as pp,
            tc.tile_pool(name="pp4", bufs=2, space="PSUM") as pp4,
        ):
            # ---- weights / biases, loaded once
            # fp8 path: hidden layers use DoubleRow -- the stationary operand
            # is a 3D [128, 2, M] tile holding k-pairs (logical k = i*128+p),
            # gathered straight from the row-major DRAM weights by the DMA.
            w0_t, wl_t, w4_t = {}, {}, {}
            w0dr_t, w0k2_t, wldr_t = {}, {}, None
            if is_fp8:
                for m in range(MT):
                    ms = slice(m * 128, (m + 1) * 128)
                    t = wp.tile([128, 2, 144], mmdt, tag=f"w0dr_{m}")
                    nc.scalar.dma_start(
                        t[:, :, 0:128],
                        w0_d.ap()[0:256, ms].rearrange("(i p) m -> p i m", p=128),
                    )
                    w0dr_t[m] = t
                    t2 = wp.tile([128, 128], mmdt, tag=f"w0k2_{m}")
                    nc.scalar.dma_start(t2[:], w0_d.ap()[256:384, ms])
                    w0k2_t[m] = t2
                wldr_t = {}
                for li, l in enumerate((1, 2, 3)):
                    for m in range(MT):
                        ms = slice(m * 128, (m + 1) * 128)
                        t = wp.tile([128, 2, 144], mmdt, tag=f"w{l}dr_{m}")
                        nc.scalar.dma_start(
                            t[:, :, 0:128],
                            wl_d[li].ap()[:, ms].rearrange(
                                "(i p) m -> p i m", p=128
                            ),
                        )
                        wldr_t[l, m] = t
                w4dr = wp.tile([128, 2, 48], mmdt, tag="w4dr")
                nc.scalar.dma_start(
                    w4dr[:, :, 0:OUT], w4_d.ap().rearrange("(i p) m -> p i m", p=128)
                )
            else:
                for k in range(KT0):
                    for m in range(MT):
                        t = wp.tile([128, 128], mmdt, tag=f"w0_{k}_{m}")
                        nc.scalar.dma_start(
                            t[:],
                            w0_d.ap()[k * 128:(k + 1) * 128, m * 128:(m + 1) * 128],
                        )
                        w0_t[k, m] = t
                for li, l in enumerate((1, 2, 3)):
                    for k in range(KTH):
                        for m in range(MT):
                            t = wp.tile([128, 128], mmdt, tag=f"w{l}_{k}_{m}")
                            nc.scalar.dma_start(
                                t[:],
                                wl_d[li].ap()[
                                    k * 128:(k + 1) * 128, m * 128:(m + 1) * 128
                                ],
                            )
                            wl_t[l, k, m] = t
                for k in range(KTH):
                    t = wp.tile([128, OUT], mmdt, tag=f"w4_{k}")
                    nc.scalar.dma_start(t[:], w4_d.ap()[k * 128:(k + 1) * 128, :])
                    w4_t[k] = t
            bl_t = {}
            if not zero_bias:
                for l in range(4):
                    t = wp.tile([128, MT], f32, tag=f"b{l}")
                    nc.scalar.dma_start(t[:], bl_d[l].ap()[:])
                    bl_t[l] = t
            # bias broadcast to both m-tiles' column ranges for fused relu:
            # fused op covers (128, MT*N); bias AP must be per-partition, so
            # we keep per-m bias and slice the fused tile per m only for the
            # bias application -- i.e. still per-m ops. Instead we fuse by
            # applying relu over the 3D psum tile per m with one op each but
            # batching both m psum banks in one tile for scheduling locality.
            b4_t = wp.tile([OUT, 1], f32, tag="b4")
            nc.scalar.dma_start(b4_t[:], b4_d.ap()[:])

            # ---- main loop over repeats x pairs of 512-row blocks.
            # Two blocks are interleaved layer-by-layer so the PE always has
            # an independent matmul stream while the other block's relu
            # drains; per-k x DMAs let L0 start on the first k-tile.
            import contextlib

            loop_cm = (
                tc.For_i(0, loop_repeat, 1) if loop_repeat
                else contextlib.nullcontext()
            )
            with loop_cm:
                for rep in range(repeat):
                    for pair in range(NBLK // grp):
                            blks = tuple(range(grp * pair, grp * (pair + 1)))
                            xts = {}
                            for b in blks:
                                xt = xp.tile([128, KT0, N], mmdt, tag=f"x{b % grp}")
                                for k in range(KT0):
                                    nc.sync.dma_start(
                                        xt[:, k, :],
                                        xt_v[:, k, b * N:(b + 1) * N],
                                    )
                                xts[b] = xt

                            h_prev = {b: None for b in blks}
                            for l in range(4):
                                w_tiles = {} if is_fp8 else (
                                    w0_t if l == 0 else {
                                        (k, m): wl_t[l, k, m]
                                        for k in range(KTH) for m in range(MT)
                                    }
                                )
                                nk = KT0 if l == 0 else KTH
                                h_cur = {}
                                pss = {}
                                for b in blks:
                                    h_cur[b] = hp.tile(
                                        [128, MT, N], mmdt,
                                        name=f"hc{l}{b % grp}",
                                        tag=f"h{l}{b % grp}",
                                    )
                                    for m in range(MT):
                                        pss[b, m] = pp.tile(
                                            [128, N], f32, name="psb", tag="ps"
                                        )
                                for b in blks:
                                    for m in range(MT):
                                        ps = pss[b, m]
                                        if is_fp8 and l == 0:
                                            nc.tensor.matmul(
                                                ps[:], w0dr_t[m][:, :, 0:128],
                                                xts[b][:, 0:2, :],
                                                start=True, stop=False,
                                                perf_mode=DR,
                                            )
                                            nc.tensor.matmul(
                                                ps[:], w0k2_t[m][:],
                                                xts[b][:, 2, :],
                                                start=False, stop=True,
                                            )
                                        elif is_fp8:
                                            nc.tensor.matmul(
                                                ps[:],
                                                wldr_t[l, m][:, :, 0:128],
                                                h_prev[b][:, :, :],
                                                start=True, stop=True,
                                                perf_mode=DR,
                                            )
                                        else:
                                            for k in range(nk):
                                                rhs = (
                                                    xts[b][:, k, :] if l == 0
                                                    else h_prev[b][:, k, :]
                                                )
                                                nc.tensor.matmul(
                                                    ps[:],
                                                    w_tiles[k, m][:],
                                                    rhs,
                                                    start=(k == 0),
                                                    stop=(k == nk - 1),
                                                )
                                        # relu as soon as this m-group stops;
                                        # engines alternate for balance
                                        if zero_bias:
                                            if (l + b + m) % 2 == 0:
                                                nc.scalar.activation(
                                                    h_cur[b][:, m, :], ps[:],
                                                    RELU,
                                                )
                                            else:
                                                nc.vector.tensor_scalar(
                                                    h_cur[b][:, m, :], ps[:],
                                                    0.0, None, MAX,
                                                )
                                        else:
                                            if (l + b + m) % 2 == 0:
                                                nc.scalar.activation(
                                                    h_cur[b][:, m, :], ps[:],
                                                    RELU,
                                                    bias=bl_t[l][:, m:m + 1],
                                                )
                                            else:
                                                nc.vector.tensor_scalar(
                                                    h_cur[b][:, m, :], ps[:],
                                                    bl_t[l][:, m:m + 1], 0.0,
                                                    ADD, MAX,
                                                )
                                for b in blks:
                                    h_prev[b] = h_cur[b]

                            for b in blks:
                                ps4 = pp4.tile([OUT, N], f32, tag="ps4")
                                if is_fp8:
                                    nc.tensor.matmul(
                                        ps4[:], w4dr[:, :, 0:OUT], h_prev[b][:, :, :],
                                        start=True, stop=True, perf_mode=DR,
                                    )
                                else:
                                    for k in range(KTH):
                                        nc.tensor.matmul(
                                            ps4[:],
                                            w4_t[k][:],
                                            h_prev[b][:, k, :],
                                            start=(k == 0),
                                            stop=(k == KTH - 1),
                                        )
                                ob = op.tile([OUT, N], f32, tag="ob")
                                if b % 2 == 0:
                                    nc.scalar.activation(
                                        ob[:], ps4[:], IDENT, bias=b4_t[:]
                                    )
                                else:
                                    nc.vector.tensor_scalar(
                                        ob[:], ps4[:], b4_t[:], None, ADD,
                                    )
                                nc.sync.dma_start(
                                    out_d.ap()[:, b * N:(b + 1) * N], ob[:]
                                )

    nc.compile()
    return nc


def _fold_weights(W_text, b_text, W_gnn, b_gnn, W_out, b_out, adjacency, template):
    s_rows = adjacency.astype(np.float64).sum(axis=1)
    if np.ptp(s_rows) > 1e-5:
        raise ValueError("adjacency row sums are not uniform; collapse invalid")
    s = float(s_rows.mean())

    W0c = (W_text.astype(np.float64) @ (s * W_gnn[0].astype(np.float64)))
    b0c = s * (b_text.astype(np.float64) @ W_gnn[0].astype(np.float64)) + b_gnn[0]
    Wl = [s * W_gnn[l].astype(np.float64) for l in (1, 2, 3)]
    bl = [b_gnn[l] for l in (1, 2, 3)]
    W4c = np.tile(W_out, (1, 12))
    b4c = np.tile(b_out, 12) + template.reshape(36)

    mdt = _np_mm_dtype()

    def cvt(a, dt):
        return np.ascontiguousarray(np.asarray(a, dtype=np.float32).astype(dt))

    biases = [
        cvt(np.asarray(b, dtype=np.float64).reshape(MT, 128).T, np.float32)
        for b in [b0c, *bl]
    ]
    return (
        cvt(W0c, mdt), [cvt(w, mdt) for w in Wl], cvt(W4c, mdt),
        biases, cvt(np.asarray(b4c).reshape(OUT, 1), np.float32),
    )


def _make_in_maps(inputs):
    x = np.asarray(inputs["text_emb"], dtype=np.float32)
    W0c, Wl, W4c, biases, b4c = _fold_weights(
        np.asarray(inputs["W_text"]), np.asarray(inputs["b_text"]),
        np.asarray(inputs["W_gnn"]), np.asarray(inputs["b_gnn"]),
        np.asarray(inputs["W_out"]), np.asarray(inputs["b_out"]),
        np.asarray(inputs["adjacency"]), np.asarray(inputs["template"]),
    )
    zero_bias = all(np.all(b == 0.0) for b in biases)
    _BUILT.setdefault("zero_bias", zero_bias)
    mdt = _np_mm_dtype()
    in_maps = []
    for c in range(CORES):
        shard = np.ascontiguousarray(x[c * ROWS:(c + 1) * ROWS].T).astype(mdt)
        m = {"xt": shard, "w0": W0c, "w4": W4c, "b4": b4c}
        for i, l in enumerate((1, 2, 3)):
            m[f"w{l}"] = Wl[i]
        if not _BUILT["zero_bias"]:
            for l in range(4):
                m[f"b{l}"] = biases[l]
        in_maps.append(m)
    return in_maps


def kernel(**inputs):
    from concourse.bass_utils import run_bass_kernel_spmd

    in_maps = _make_in_maps(inputs)
    if "nc" not in _BUILT:
        _BUILT["nc"] = _build_bass(repeat=1)
    nc = _BUILT["nc"]
    res = run_bass_kernel_spmd(nc, in_maps, core_ids=list(range(CORES)))
    _BUILT["last_results"] = res
    _BUILT["last_in_maps"] = in_maps

    full = np.empty((B, OUT), dtype=np.float32)
    for c in range(CORES):
        full[c * ROWS:(c + 1) * ROWS] = res.results[c]["out"].T
    return full.reshape(B, 12, 3)



# revision 12
# speedup vs baseline: 1.1059x; 1.1059x over previous
"""MeshGNN Trainium2 kernel (fp8 DoubleRow + 3-engine relu).

Mathematical reduction: the reference broadcasts the text projection to all 12
mesh vertices, and the row-normalized kNN adjacency has identical row sums
(every vertex has exactly K_NN=6 neighbors), so node features stay identical
across vertices through every GNN layer.  The whole network collapses to a
per-row MLP:

    h   = relu(x @ W0c)            W0c = W_text @ (s*W_gnn[0])  (384,256)
    h   = relu(h @ (s*W_gnn[l]))   l = 1..3
    o36 = h @ W4c                  W4c = tile(W_out, 12) (256,36)
    out = o36.reshape(B, 12, 3) + b4c   (host adds b4c = tiled b_out + template)

(s = 6/(6+1e-6); all layer biases are zero for this problem's inputs --
checked at fold time, with a per-m bias fallback if they ever aren't.)

Device design (8 cores, data parallel over batch; per core 4096 rows):
  - all matmuls fp8e4 in DoubleRow perf mode (0.5 PE cycles/row).  L0's
    K=384 is host-padded to 512 with zeros so both k-pairs run DR.
  - weights live in ONE host-packed [128, 22, 144] fp8 SBUF image loaded
    with a single DMA; DR stationary operands are strided views into it.
  - the bottleneck is the elementwise relu (4 layers x 256 x 4096 elems):
    one fused op per (block, layer) over the [128, 2, 512] PSUM pair-tile,
    spread over the three elementwise engines (Pool 13 / ACT 10 / DVE 9),
    all 8 blocks software-pipelined.
  - PSUM: one unified ring of 4 x [128,2,512] f32 tiles = all 8 banks.
  - x is host-packed block-major so every DMA moves 4KB/partition runs
    (128 descriptors); output pairs land at PSUM partitions 0:36 / 64:100
    and are DMA'd straight from PSUM (template/bias added on host).
"""

import numpy as np

# ---------------------------------------------------------------- constants
B = 32768
CORES = 8
ROWS = B // CORES            # 4096 rows per core
TD = 384                     # text dim
KPAD = 512                   # L0 contraction padded to 4 k-tiles
H = 256                      # hidden
OUT = 36                     # 12 verts * 3 coords
NBLK = 8                     # row blocks per core
N = ROWS // NBLK             # 512 rows per block
OBP = 64 + OUT               # output partitions (odd block at base 64)
NW = 22                      # packed weight chunks of [128, 144]

# relu engine schedule [layer][block]: A=ACT, D=DVE (17/15).
# GPSIMD cannot read PSUM on TRN2, so only these two engines can drain it.
RELU_ENG = (
    "ADADADAD",
    "DADADADA",
    "ADADADAA",
    "ADADADAD",
)

_BUILT = {}                  # cache: compiled Bass modules keyed by config


def _fp8_np():
    import concourse.mybir as mybir
    return mybir.dt.np(mybir.dt.float8e4)


def _build_bass(repeat=1, loop_repeat=0, zero_bias=None):
    """Build + compile the per-core Bass program (same NEFF on all cores).

    loop_repeat > 0 wraps the pipeline in a device-side For_i loop executed
    that many times (identical outputs; ~2us barrier per back-edge) -- used
    for timing with enough device work to swamp dispatch noise entirely.
    """
    import contextlib

    import concourse.mybir as mybir
    import concourse.tile as tile
    from concourse import bacc

    if zero_bias is None:
        zero_bias = _BUILT.get("zero_bias", True)

    f32 = mybir.dt.float32
    fp8 = mybir.dt.float8e4
    DR = mybir.MatmulPerfMode.DoubleRow
    RELU = mybir.ActivationFunctionType.Relu
    ADD = mybir.AluOpType.add
    MAX = mybir.AluOpType.max

    nc = bacc.Bacc(
        "TRN2",
        target_bir_lowering=False,
        debug=False,
        enable_asserts=False,
        num_devices=CORES,
    )

    # x block-major: row p holds, per block b, the 4 k-tiles' 512 columns
    xt_d = nc.dram_tensor("xt", (128, NBLK * 4 * N), fp8, kind="ExternalInput")
    w_d = nc.dram_tensor("wpk", (128, NW * 144), fp8, kind="ExternalInput")
    bl_d = None if zero_bias else [
        nc.dram_tensor(f"b{l}", (128, 2), f32, kind="ExternalInput")
        for l in range(4)
    ]
    out_d = nc.dram_tensor(
        "out", (OUT, ROWS), mybir.dt.bfloat16, kind="ExternalOutput"
    )

    xt_v = xt_d.ap().rearrange("p (b k n) -> p b k n", k=4, n=N)
    out_v = out_d.ap().rearrange("p (b n) -> p b n", n=N)

    # packed-weight chunk index for each DR stationary operand
    ch_l0 = lambda m, pair: 4 * m + 2 * pair          # noqa: E731
    ch_l = lambda l, m: 8 + 4 * (l - 1) + 2 * m       # noqa: E731
    CH_L4 = 20

    with tile.TileContext(nc) as tc:
        with (
            tc.tile_pool(name="wp", bufs=1) as wp,
            tc.tile_pool(name="xp", bufs=1) as xp,
            tc.tile_pool(name="hp", bufs=2) as hp,
            tc.tile_pool(name="op", bufs=4) as op,
            tc.tile_pool(name="pp", bufs=4, space="PSUM") as pp,
        ):
            # ---- weights / biases: one packed image, loaded once
            wsb = wp.tile([128, NW, 144], fp8, tag="w")
            nc.sync.dma_start(
                wsb[:, :, :],
                w_d.ap().rearrange("p (a b) -> p a b", b=144),
            )
            blt = {}
            if not zero_bias:
                for l in range(4):
                    t = wp.tile([128, 2], f32, tag=f"b{l}")
                    nc.sync.dma_start(t[:], bl_d[l].ap()[:])
                    blt[l] = t

            xt = xp.tile([128, NBLK, 4, N], fp8, tag="x")

            # dummy 1-elem activation before the loop: forces the Relu/Ident
            # ACT table load to happen once at startup, not inside For_i
            warm = wp.tile([1, 1], f32, tag="warm")
            nc.scalar.activation(warm[:], warm[:], RELU)

            def relu(l, b, dst, src):
                if RELU_ENG[l][b] == "A":
                    nc.scalar.activation(dst, src, RELU)
                else:
                    nc.vector.tensor_scalar(dst, src, 0.0, None, MAX)

            COPY_ENG = ("D", "A", "D", "A")

            def out_copy(pr, dst, src):
                if COPY_ENG[pr % 4] == "A":
                    nc.scalar.activation(
                        dst, src, mybir.ActivationFunctionType.Identity
                    )
                else:
                    nc.vector.tensor_scalar(dst, src, 0.0, None, ADD)

            def relu_bias(l, b, m, dst, src, bias_ap):
                if RELU_ENG[l][b] == "A":
                    nc.scalar.activation(dst, src, RELU, bias=bias_ap)
                else:
                    nc.vector.tensor_scalar(dst, src, bias_ap, 0.0, ADD, MAX)

            loop_cm = (
                tc.For_i(0, loop_repeat, 1) if loop_repeat
                else contextlib.nullcontext()
            )
            with loop_cm:
                for rep in range(repeat):
                    # input: 4 DMAs (1,2,2,3 blocks): first block lands
                    # sooner; later blocks stream in ahead of the PE
                    for lo, hi in ((0, 1), (1, 3), (3, 5), (5, 8)):
                        nc.sync.dma_start(
                            xt[:, lo:hi, :, :],
                            xt_v[:, lo:hi, :, :],
                        )

                    def emit_pair(pr):
                        be, bo = 2 * pr, 2 * pr + 1
                        ps4 = pp.tile([128, 2, N], f32, tag="ps")
                        nc.tensor.matmul(
                            ps4[0:OUT, 0, :],
                            wsb[:, CH_L4:CH_L4 + 2, 0:OUT],
                            h_prev[be][:, 0:2, :],
                            start=True, stop=True, perf_mode=DR,
                        )
                        nc.tensor.matmul(
                            ps4[0:OUT, 1, :],
                            wsb[:, CH_L4:CH_L4 + 2, 0:OUT],
                            h_prev[bo][:, 0:2, :],
                            start=True, stop=True, perf_mode=DR,
                        )
                        ob = op.tile([OUT, 2, N], mybir.dt.bfloat16, tag="ob")
                        out_copy(pr, ob[:, :, :], ps4[0:OUT, 0:2, :])
                        nc.sync.dma_start(
                            out_v[:, 2 * pr:2 * pr + 2, :],
                            ob[:, :, :],
                        )

                    h_prev = {}
                    for l in range(4):
                        for b in range(NBLK):
                            ps = pp.tile([128, 2, N], f32, tag="ps")
                            h = hp.tile(
                                [128, 2, N], fp8,
                                name=f"h{l}{b}", tag=f"h{b}",
                            )
                            if l == 0:
                                for m in range(2):
                                    c0, c1 = ch_l0(m, 0), ch_l0(m, 1)
                                    nc.tensor.matmul(
                                        ps[:, m, :],
                                        wsb[:, c0:c0 + 2, 0:128],
                                        xt[:, b, 0:2, :],
                                        start=True, stop=False,
                                        perf_mode=DR,
                                    )
                                    nc.tensor.matmul(
                                        ps[:, m, :],
                                        wsb[:, c1:c1 + 2, 0:128],
                                        xt[:, b, 2:4, :],
                                        start=False, stop=True,
                                        perf_mode=DR,
                                    )
                            else:
                                for m in range(2):
                                    c = ch_l(l, m)
                                    nc.tensor.matmul(
                                        ps[:, m, :],
                                        wsb[:, c:c + 2, 0:128],
                                        h_prev[b][:, 0:2, :],
                                        start=True, stop=True,
                                        perf_mode=DR,
                                    )
                            if zero_bias:
                                relu(l, b, h[:, :, :], ps[:, :, :])
                            else:
                                for m in range(2):
                                    relu_bias(
                                        l, b, m, h[:, m, :], ps[:, m, :],
                                        blt[l][:, m:m + 1],
                                    )
                            h_prev[b] = h
                            if l == 3 and b % 2 == 1:
                                emit_pair(b // 2)

    nc.compile()
    return nc


def _fold_weights(W_text, b_text, W_gnn, b_gnn, W_out, b_out, adjacency, template):
    s_rows = adjacency.astype(np.float64).sum(axis=1)
    if np.ptp(s_rows) > 1e-5:
        raise ValueError("adjacency row sums are not uniform; collapse invalid")
    s = float(s_rows.mean())

    W0c = (W_text.astype(np.float64) @ (s * W_gnn[0].astype(np.float64)))
    b0c = s * (b_text.astype(np.float64) @ W_gnn[0].astype(np.float64)) + b_gnn[0]
    Wl = [s * W_gnn[l].astype(np.float64) for l in (1, 2, 3)]
    bl = [b_gnn[l] for l in (1, 2, 3)]
    W4c = np.tile(W_out, (1, 12))
    b4c = np.tile(b_out, 12) + template.reshape(OUT)
    biases = [np.asarray(b, dtype=np.float32) for b in [b0c, *bl]]
    return W0c, Wl, W4c, biases, np.asarray(b4c, dtype=np.float32)


def _pack_weights(W0c, Wl, W4c):
    """Pack all matmul weights into the [128, NW, 144] fp8 SBUF image.

    Chunk pairs (c, c+1) hold a DR stationary operand: element (p, i, m) of
    view [:, c:c+2, 0:M] must equal W[pair_k0*128 + i*128 + p, m]."""
    fp8 = _fp8_np()
    img = np.zeros((128, NW, 144), dtype=fp8)

    def put(c, Wsub):                      # Wsub: (256, M) fp8
        M = Wsub.shape[1]
        img[:, c, :M] = Wsub[0:128]
        img[:, c + 1, :M] = Wsub[128:256]

    W0p = np.zeros((KPAD, H), dtype=fp8)
    W0p[0:TD] = W0c.astype(np.float32).astype(fp8)
    Wlq = [w.astype(np.float32).astype(fp8) for w in Wl]
    W4q = W4c.astype(np.float32).astype(fp8)

    for m in range(2):
        ms = slice(m * 128, (m + 1) * 128)
        put(4 * m + 0, W0p[0:256, ms])
        put(4 * m + 2, W0p[256:512, ms])
    for li in range(3):
        for m in range(2):
            put(8 + 4 * li + 2 * m, Wlq[li][:, m * 128:(m + 1) * 128])
    put(20, W4q)
    return np.ascontiguousarray(img.reshape(128, NW * 144))


def _make_in_maps(inputs):
    x = np.asarray(inputs["text_emb"], dtype=np.float32)
    W0c, Wl, W4c, biases, b4c = _fold_weights(
        np.asarray(inputs["W_text"]), np.asarray(inputs["b_text"]),
        np.asarray(inputs["W_gnn"]), np.asarray(inputs["b_gnn"]),
        np.asarray(inputs["W_out"]), np.asarray(inputs["b_out"]),
        np.asarray(inputs["adjacency"]), np.asarray(inputs["template"]),
    )
    zero_bias = all(np.all(b == 0.0) for b in biases)
    _BUILT.setdefault("zero_bias", zero_bias)
    _BUILT["b4c"] = b4c
    fp8 = _fp8_np()
    wimg = _pack_weights(W0c, Wl, W4c)
    in_maps = []
    for c in range(CORES):
        xpad = np.zeros((KPAD, ROWS), dtype=fp8)
        xpad[0:TD] = np.ascontiguousarray(
            x[c * ROWS:(c + 1) * ROWS].T
        ).astype(fp8)
        # block-major pack: (p, b, k, j) = xpad[k*128 + p, b*N + j]
        xb = np.ascontiguousarray(
            xpad.reshape(4, 128, NBLK, N).transpose(1, 2, 0, 3)
        ).reshape(128, NBLK * 4 * N)
        m = {"xt": xb, "wpk": wimg}
        if not _BUILT["zero_bias"]:
            for l in range(4):
                m[f"b{l}"] = np.ascontiguousarray(
                    biases[l].reshape(2, 128).T.astype(np.float32)
                )
        in_maps.append(m)
    return in_maps


def kernel(**inputs):
    from concourse.bass_utils import run_bass_kernel_spmd

    in_maps = _make_in_maps(inputs)
    if "nc" not in _BUILT:
        _BUILT["nc"] = _build_bass(repeat=1)
    nc = _BUILT["nc"]
    res = run_bass_kernel_spmd(nc, in_maps, core_ids=list(range(CORES)))
    _BUILT["last_results"] = res
    _BUILT["last_in_maps"] = in_maps

    b4c = _BUILT["b4c"]
    full = np.empty((B, OUT), dtype=np.float32)
    for c in range(CORES):
        o = np.asarray(
            res.results[c]["out"], dtype=np.float32
        ).reshape(OUT, ROWS)
        full[c * ROWS:(c + 1) * ROWS] = o.T
    full += b4c[None, :]
    return full.reshape(B, 12, 3)
